# revision 15
# baseline (speedup 1.0000x reference)
"""AdaptiveECE Trainium2 kernel (8 NeuronCores, SPMD).

Device (per core, rows sharded 8-way): stream the [32768, 1000] f32 logits
shard through SBUF; VectorE computes per-row max, ScalarE computes
exp + per-row sum (accum_out) in the same streaming pass.  Only 2 floats
per row leave the device.

Host: conf = exp(m)/S, acc = (logits[i, label_i] == m_i), exact
equal-count quantile edges from sorted conf, ECE via the reference's
binning semantics.
"""

import contextlib
import ctypes
import os
import sys
import types

sys.path.insert(0, "/opt/trn_rl_repo")

import numpy as np

N = 262144
C = 1000
NCORES = 8
NBINS = 15
ROWS_PER_CORE = N // NCORES          # 32768
NCOLS = ROWS_PER_CORE // 128         # 256 row-groups of 128 rows per core
RING = 40                            # SBUF ring depth in row-group slots

# chunk schedule (in row-groups of 128 rows / 512 KB): small chunks at the
# start (fast DMA ramp) and end (short compute tail), 8-group (4 MB) chunks
# in the middle.  Must sum to NCOLS and never cross the RING boundary.
CHUNKS = [1, 1, 2, 4] + [8] * 30 + [4, 2, 1, 1]
assert sum(CHUNKS) == NCOLS
_s = 0
for _k in CHUNKS:
    assert (_s % RING) + _k <= RING, (_s, _k)
    _s += _k

LAST_EXEC_NS = None
LAST_TRACE_DIR = None

_GRAPH = None


def _install_ntff_shim():
    """Provide antenv.axon_hooks (missing in this image) so
    run_bass_kernel_spmd(trace=True) can NTFF-profile via libaxon_pjrt."""
    if "antenv.axon_hooks" in sys.modules:
        return
    so_path = "/opt/axon/libaxon_pjrt.so"
    hook = None
    try:
        lib = ctypes.CDLL(so_path)
        if hasattr(lib, "axon_start_nrt_profile"):
            lib.axon_start_nrt_profile.argtypes = [
                ctypes.POINTER(ctypes.c_int64),
                ctypes.c_size_t,
            ]
            lib.axon_start_nrt_profile.restype = ctypes.c_int64
            lib.axon_stop_nrt_profile.argtypes = [ctypes.c_char_p]
            lib.axon_stop_nrt_profile.restype = ctypes.c_int64

            @contextlib.contextmanager
            def _hook(output_dir, device_ids):
                import jax

                jax.devices()
                if device_ids:
                    ids = (ctypes.c_int64 * len(device_ids))(*device_ids)
                    rc = lib.axon_start_nrt_profile(ids, len(device_ids))
                else:
                    rc = lib.axon_start_nrt_profile(None, 0)
                if rc != 0:
                    raise RuntimeError(f"axon_start_nrt_profile rc={rc}")
                try:
                    yield
                finally:
                    n = lib.axon_stop_nrt_profile(str(output_dir).encode())
                    print(f"profile: {n} file(s) -> {output_dir}", file=sys.stderr)

            hook = _hook
    except OSError:
        pass
    mod = types.ModuleType("antenv.axon_hooks")
    mod.get_axon_ntff_profile_hook = lambda: hook
    mod.set_axon_ntff_profile_hook = lambda h: None
    sys.modules["antenv.axon_hooks"] = mod


def _build_graph():
    global _GRAPH
    if _GRAPH is not None:
        return _GRAPH

    import concourse.bass as bass
    import concourse.mybir as mybir

    f32 = mybir.dt.float32
    nc = bass.Bass()
    x = nc.declare_dram_parameter("x", [ROWS_PER_CORE, C], f32, isOutput=False)
    m_ext = nc.declare_dram_parameter("m", [128, NCOLS], f32, isOutput=True)
    s_ext = nc.declare_dram_parameter("s", [128, NCOLS], f32, isOutput=True)

    # contiguous-stripe layout: partition p owns rows [p*NCOLS, (p+1)*NCOLS);
    # row-group j = rows {p*NCOLS + j}; a chunk [a, a+k) is one contiguous
    # 4000*k-byte DRAM run per partition (cheap HWDGE descriptor generation).
    xg = x.rearrange("(p r) c -> p r c", p=128)

    nchunks = len(CHUNKS)
    starts = []
    _a = 0
    for k in CHUNKS:
        starts.append(_a)
        _a += k

    # which chunk previously occupied each ring slot (for buffer reuse waits)
    slot_owner = [None] * RING

    with (
        nc.sbuf_tensor([128, RING, C], f32) as ltile,
        nc.psum_tensor([128, C], f32) as etile,
        nc.sbuf_tensor([128, NCOLS], f32) as m_buf,
        nc.psum_tensor([128, NCOLS], f32) as s_psum,
        nc.sbuf_tensor([128, NCOLS], f32) as s_buf,
        nc.semaphore("dma_sem") as dma_sem,
        nc.semaphore("v_sem") as v_sem,
        nc.semaphore("a_sem") as a_sem,
        nc.Block() as block,
    ):

        @block.sync
        def _(sync):
            for i, (a, k) in enumerate(zip(starts, CHUNKS)):
                s = a % RING
                need = None
                for j in range(s, s + k):
                    if slot_owner[j] is not None:
                        need = (
                            slot_owner[j]
                            if need is None
                            else max(need, slot_owner[j])
                        )
                    slot_owner[j] = i
                if need is not None:
                    sync.wait_ge(v_sem, need + 1)
                    sync.wait_ge(a_sem, need + 1)
                sync.dma_start(
                    out=ltile[:, s : s + k, :], in_=xg[:, a : a + k, :]
                ).then_inc(dma_sem, 16)
            sync.wait_ge(v_sem, nchunks)
            sync.dma_start(out=m_ext[:], in_=m_buf[:]).then_inc(dma_sem, 16)
            sync.wait_ge(a_sem, nchunks + 1)
            sync.dma_start(out=s_ext[:], in_=s_buf[:]).then_inc(dma_sem, 16)
            sync.wait_ge(dma_sem, 16 * (nchunks + 2))

        @block.vector
        def _(vector):
            for i, (a, k) in enumerate(zip(starts, CHUNKS)):
                s = a % RING
                vector.wait_ge(dma_sem, 16 * (i + 1))
                nc.vector.reduce_max(
                    m_buf[:, a : a + k],
                    ltile[:, s : s + k, :],
                    axis=mybir.AxisListType.X,
                ).then_inc(v_sem, 1)

        @block.scalar
        def _(scalar):
            # dummy exp to pull ACT_TABLE_LOAD into the DMA ramp shadow
            nc.scalar.activation(
                etile[:, :1], etile[:, :1], mybir.ActivationFunctionType.Exp
            )
            for i, (a, k) in enumerate(zip(starts, CHUNKS)):
                s = a % RING
                scalar.wait_ge(dma_sem, 16 * (i + 1))
                for g in range(k):
                    ins = nc.scalar.activation(
                        etile[:],
                        ltile[:, s + g, :],
                        mybir.ActivationFunctionType.Exp,
                        accum_out=s_psum[:, a + g : a + g + 1],
                    )
                    if g == k - 1:
                        ins.then_inc(a_sem, 1)
            # drain accumulated sums to SBUF so DMA can reach them
            nc.scalar.copy(s_buf[:], s_psum[:]).then_inc(a_sem, 1)

    _GRAPH = nc
    return nc


def _cols_to_rows(a):
    # a[p, j] with j = GROUPS*t + g  ->  local row p*NCOLS + j
    return a.reshape(-1)


def _run_device(logits):
    global LAST_EXEC_NS, LAST_TRACE_DIR
    _install_ntff_shim()
    from concourse.bass_utils import run_bass_kernel_spmd

    nc = _build_graph()
    trace = bool(os.environ.get("KERNEL_TRACE"))
    in_maps = [
        {"x": np.ascontiguousarray(logits[c * ROWS_PER_CORE : (c + 1) * ROWS_PER_CORE])}
        for c in range(NCORES)
    ]
    try:
        res = run_bass_kernel_spmd(
            nc, in_maps, core_ids=list(range(NCORES)), trace=trace
        )
    except Exception:
        # transient device/tunnel failure: rebuild graph once and retry
        global _GRAPH
        _GRAPH = None
        nc = _build_graph()
        res = run_bass_kernel_spmd(
            nc, in_maps, core_ids=list(range(NCORES)), trace=trace
        )
    LAST_EXEC_NS = res.exec_time_ns
    m = np.concatenate([_cols_to_rows(res.results[c]["m"]) for c in range(NCORES)])
    s = np.concatenate([_cols_to_rows(res.results[c]["s"]) for c in range(NCORES)])
    return m, s


def kernel(logits, labels):
    logits = np.asarray(logits, dtype=np.float32)
    labels = np.asarray(labels)

    m, s = _run_device(logits)

    conf = (np.exp(m.astype(np.float64)) / s.astype(np.float64)).astype(np.float32)
    g = logits[np.arange(N), labels]
    acc = (g == m).astype(np.float64)

    conf64 = conf.astype(np.float64)
    sc = np.sort(conf64)
    xq = np.linspace(0.0, float(N), NBINS + 1)
    edges = np.interp(xq, np.arange(N, dtype=np.float64), sc)

    bin_id = np.searchsorted(edges[1:], conf64, side="left")
    bin_id = np.clip(bin_id, 0, NBINS - 1)
    valid = conf64 > edges[0]

    bv = bin_id[valid]
    counts = np.bincount(bv, minlength=NBINS).astype(np.float64)
    sum_acc = np.bincount(bv, weights=acc[valid], minlength=NBINS)
    sum_conf = np.bincount(bv, weights=conf64[valid], minlength=NBINS)

    nonempty = counts > 0
    denom = np.maximum(counts, 1.0)
    ece = np.sum(
        np.where(
            nonempty,
            np.abs(sum_conf / denom - sum_acc / denom) * (counts / float(N)),
            0.0,
        )
    )
    return np.asarray([ece], dtype=np.float32)
